# revision 22
# baseline (speedup 1.0000x reference)
"""CrossAttention Trainium2 kernel (8 NeuronCores, SPMD).

Problem: x [4,256,64,64], context [4,512,32,32], 8 heads x 64 dim,
q = Wq@x, k = Wk@ctx, v = Wv@ctx, attn = softmax(q^T k / 8), out = Wo@(v attn^T) + bo.

Sharding: fully data-parallel over (batch, query-spatial-half) -> 8 shards.
Each core computes K/V for its batch (duplicated per pair) and attention +
output projection for its 2048 query positions. Zero collectives.

Pipeline (ScalarE exp is the roofline engine at ~110us/core; everything is
scheduled so ACT never stalls):
  - dummy PE warm-up burst during the input DMA so HAM unthrottles before the
    first projection;
  - first exp issues after only Kproj(hp0)+Qproj(hp0,it0)+1 sim (~7us); all
    other K/V/Q projections stream through a 1-bank PSUM pool inside it0's
    PE slack;
  - simT[j,i] = k^T q per head-pair (two K=64 matmuls packed via
    tile_position); exp on ScalarE out of PSUM; AV accumulation with M=65
    (ones column = softmax denominator);
  - per-(it,hp) normalization is a 4-stage pipeline (DVE drain -> DRAM
    bounce -> reciprocal -> stride-0 partition-broadcast -> multiply) whose
    stages are emitted 1-2 blocks later in the DVE stream so no DVE
    instruction ever waits in-queue;
  - the output projection for block b is emitted at block b+3 (ob-major,
    single PSUM bank), so the kernel tail is just one normalization chain.
"""
import os
import sys
import numpy as np

for _p in ("/opt/trn_rl_repo", "/root/.axon_site/_ro/trn_rl_repo"):
    if os.path.isdir(_p) and _p not in sys.path:
        sys.path.insert(0, _p)

import concourse.bass as bass
import concourse.mybir as mybir
from concourse.tile import TileContext
from concourse.bass_utils import run_bass_kernel_spmd

F32 = mybir.dt.float32
F16 = mybir.dt.float16
EXP = mybir.ActivationFunctionType.Exp

B, H, D = 4, 8, 64
EQ, EK = 256, 512          # x channels, ctx channels
NQ, NK = 2048, 1024        # per-core query positions, kv positions
OC = 256                   # output channels
SCALE = D ** -0.5
IT, JT = NQ // 512, NK // 128   # 4 i-tiles of 512, 8 j-tiles of 128


def _split_excess_waits(nc, max_waits=1):
    """This walrus build rejects instructions carrying >max_waits sem waits;
    move the extras onto standalone nops just before (same engine, in-order,
    so semantics are unchanged)."""
    n_new = 0
    for f in nc.m.functions:
        for bb in f.blocks:
            insts = list(bb.instructions)
            out = []
            changed = False
            for inst in insts:
                si = inst.sync_info
                if si is not None and si.on_wait and len(si.on_wait) > max_waits:
                    waits = list(si.on_wait)
                    for w in waits[:-max_waits]:
                        nop = mybir.InstNoOp(
                            name=f"I-splitw-{n_new}",
                            sync_info=mybir.SyncInfo(on_wait=[w], on_update=[]),
                        )
                        nop.engine = inst.engine
                        n_new += 1
                        out.append(nop)
                        nc.register_instruction(nop, overwrite=True)
                    si.on_wait = waits[-max_waits:]
                    inst.sync_info = si
                    changed = True
                out.append(inst)
            if changed:
                bb.instructions.clear()
                bb.instructions.extend(out)
    return n_new


def _build():
    nc = bass.Bass()
    x_s = nc.declare_dram_parameter("x_s", [EQ, NQ], F16, isOutput=False)
    ctx_s = nc.declare_dram_parameter("ctx_s", [EK, NK], F16, isOutput=False)
    WqT = nc.declare_dram_parameter("WqT", [EQ, 512], F16, isOutput=False)
    WkT = nc.declare_dram_parameter("WkT", [EK, 512], F16, isOutput=False)
    WvT = nc.declare_dram_parameter("WvT", [EK, 512], F16, isOutput=False)
    WoT = nc.declare_dram_parameter("WoT", [512, OC], F16, isOutput=False)
    bo = nc.declare_dram_parameter("bo", [OC], F32, isOutput=False)
    y = nc.declare_dram_parameter("y", [OC, NQ], F32, isOutput=True)

    sscratch2 = nc.dram_tensor("sscratch2", [IT, 4, 1024], F32)

    with TileContext(nc) as tc:
        with (
            tc.tile_pool(name="consts", bufs=1) as cp,
            tc.tile_pool(name="qkv", bufs=1) as qp,
            tc.tile_pool(name="exps", bufs=16) as ep,
            tc.tile_pool(name="avrp", bufs=4) as avrp,
            tc.tile_pool(name="avnp", bufs=8) as avnp,
            tc.tile_pool(name="normp", bufs=3) as normp,
            tc.tile_pool(name="yout", bufs=2) as yop,
            tc.tile_pool(name="slab", bufs=2, space="PSUM") as slabp,
            tc.tile_pool(name="avp", bufs=1, space="PSUM") as avp,
            tc.tile_pool(name="yp", bufs=1, space="PSUM") as yp,
            tc.tile_pool(name="pp", bufs=1, space="PSUM") as pp,
        ):
            # ---- warm-up source (no DMA dep; memset only) ----
            warm_src = cp.tile([128, 512], F16, tag="warm_src")
            nc.vector.memset(warm_src, 0.25)

            # ---- DMA loads: critical-path-first order ----
            # exp0 needs only ctx + wkt[hp0] + wqt[hp0] + x[:, it0];
            # everything else streams in behind it.
            wkt = cp.tile([128, 4 * 512], F16, tag="wkt")
            wvt = cp.tile([128, 4 * 512], F16, tag="wvt")
            ctx_sb = cp.tile([128, 4 * NK], F16, tag="ctx_sb")
            wqt = cp.tile([128, 2 * 512], F16, tag="wqt")
            x_sb = cp.tile([128, 2 * NQ], F16, tag="x_sb")
            wot = cp.tile([128, 4 * OC], F16, tag="wot")
            bo_col = cp.tile([128, 2], F32, tag="bo_col")
            # Loads split between the sync HWDGE ring and the GpSimd SWDGE
            # ring — each ring is one in-order queue, so one ring would
            # serialize the whole 3.7MB load. ACT's ring stays clean so the
            # exp table load isn't delayed behind load triggers.
            # ctx is split by column halves: kproj ntile0 (which feeds the
            # first 4 j-tiles of sims) needs only cols 0:512 of every ec
            # chunk, so those land first on both rings.
            # sync ring: ctx ec0/ec1 h0 -> wkt/wqt/x criticals -> rest
            for ec in range(2):
                nc.sync.dma_start(out=ctx_sb[:, ec * NK:ec * NK + 512],
                                  in_=ctx_s[ec * 128:(ec + 1) * 128, 0:512])
            for ec in range(4):
                nc.sync.dma_start(out=wkt[:, ec * 512:ec * 512 + 128],
                                  in_=WkT[ec * 128:(ec + 1) * 128, 0:128])
            for ec in range(2):
                nc.sync.dma_start(out=wqt[:, ec * 512:ec * 512 + 128],
                                  in_=WqT[ec * 128:(ec + 1) * 128, 0:128])
                nc.sync.dma_start(out=x_sb[:, ec * NQ:ec * NQ + 512],
                                  in_=x_s[ec * 128:(ec + 1) * 128, 0:512])
            for ec in range(2):
                nc.sync.dma_start(out=ctx_sb[:, ec * NK + 512:(ec + 1) * NK],
                                  in_=ctx_s[ec * 128:(ec + 1) * 128, 512:NK])
            for ec in range(4):
                nc.sync.dma_start(out=wkt[:, ec * 512 + 128:(ec + 1) * 512],
                                  in_=WkT[ec * 128:(ec + 1) * 128, 128:512])
            for ec in range(2):
                nc.sync.dma_start(out=x_sb[:, ec * NQ + 512:(ec + 1) * NQ],
                                  in_=x_s[ec * 128:(ec + 1) * 128, 512:NQ])
            for ec in range(4):
                nc.sync.dma_start(out=wot[:, ec * OC:(ec + 1) * OC],
                                  in_=WoT[ec * 128:(ec + 1) * 128, :])
            for ob in range(2):
                nc.sync.dma_start(out=bo_col[:, ob:ob + 1],
                                  in_=bo[ob * 128:(ob + 1) * 128])
            # gpsimd ring: ctx ec2/ec3 (h0 first), then wvt, then wqt rest
            for ec in range(2, 4):
                nc.gpsimd.dma_start(out=ctx_sb[:, ec * NK:ec * NK + 512],
                                    in_=ctx_s[ec * 128:(ec + 1) * 128, 0:512])
            for ec in range(2, 4):
                nc.gpsimd.dma_start(out=ctx_sb[:, ec * NK + 512:(ec + 1) * NK],
                                    in_=ctx_s[ec * 128:(ec + 1) * 128, 512:NK])
            for ec in range(4):
                nc.gpsimd.dma_start(out=wvt[:, ec * 512:(ec + 1) * 512],
                                    in_=WvT[ec * 128:(ec + 1) * 128, :])
            for ec in range(2):
                nc.gpsimd.dma_start(out=wqt[:, ec * 512 + 128:(ec + 1) * 512],
                                    in_=WqT[ec * 128:(ec + 1) * 128, 128:512])

            # persistent activations
            q_sb = qp.tile([128, 4 * NQ], F16, tag="q_sb")      # [hp, i]
            k_sb = qp.tile([128, 4 * NK], F16, tag="k_sb")      # [hp, j]
            vt_sb = qp.tile([128, JT * 520], F16, tag="vt_sb")  # [jt, h*65 + c]

            # ones columns of vt (col 64 of each 65-block)
            vt_4d = vt_sb.rearrange("p (j h c) -> p j h c", j=JT, h=H)
            ones_f32 = cp.tile([128, JT * H], F32, tag="ones_f32")
            nc.vector.memset(ones_f32, 1.0)
            nc.vector.tensor_copy(
                vt_4d[:, :, :, 64:65],
                ones_f32.rearrange("p (j h) -> p j h", j=JT).unsqueeze(-1))

            # ---- PE warm-up: dummy matmuls during the DMA wait ----
            # long enough to bridge into kproj0 so HAM stays at K=8/8
            for _ in range(10):
                wps = pp.tile([128, 512], F32, tag="pps")
                nc.tensor.matmul(wps, lhsT=warm_src[:, 0:128],
                                 rhs=warm_src, start=True, stop=True)

            # ---- projection emitters ----
            # rotate projection PSUM tiles through idle banks so the
            # single-buffer drain bubble never blocks the PE stream
            _rot = {"seq": []}

            def _ptile():
                if not _rot["seq"]:
                    _rot["seq"] = [(pp, "pps")]
                pool, tag = _rot["seq"].pop(0)
                t = pool.tile([128, 512], F32, tag=tag)
                return t

            def kproj_half(hp, ntile):
                pk = _ptile()
                for ec in range(4):
                    nc.tensor.matmul(
                        pk,
                        lhsT=wkt[:, ec * 512 + hp * 128: ec * 512 + (hp + 1) * 128],
                        rhs=ctx_sb[:, ec * NK + ntile * 512: ec * NK + (ntile + 1) * 512],
                        start=(ec == 0), stop=(ec == 3))
                nc.vector.tensor_copy(
                    k_sb[:, hp * NK + ntile * 512: hp * NK + (ntile + 1) * 512], pk)

            def vproj(jt):
                pv = _ptile()
                for ec in range(4):
                    nc.tensor.matmul(
                        pv,
                        lhsT=ctx_sb[:, ec * NK + jt * 128: ec * NK + (jt + 1) * 128],
                        rhs=wvt[:, ec * 512:(ec + 1) * 512],
                        start=(ec == 0), stop=(ec == 3))
                vt_t = vt_sb[:, jt * 520:(jt + 1) * 520].rearrange(
                    "p (h c) -> p h c", h=H)[:, :, 0:64]
                nc.vector.tensor_copy(vt_t, pv.rearrange("p (h c) -> p h c", c=64))

            def qproj(hp, it):
                pq = _ptile()
                for ec in range(2):
                    nc.tensor.matmul(
                        pq,
                        lhsT=wqt[:, ec * 512 + hp * 128: ec * 512 + (hp + 1) * 128],
                        rhs=x_sb[:, ec * NQ + it * 512: ec * NQ + (it + 1) * 512],
                        start=(ec == 0), stop=(ec == 1))
                nc.vector.tensor_copy(
                    q_sb[:, hp * NQ + it * 512: hp * NQ + (it + 1) * 512], pq)

            def sim_emit(hp, it, jt):
                slab = slabp.tile([128, 1024], F32, tag="slab")
                ks = slice(hp * NK + jt * 128, hp * NK + (jt + 1) * 128)
                qs = slice(hp * NQ + it * 512, hp * NQ + (it + 1) * 512)
                nc.tensor.matmul(
                    slab[:, 0:512], lhsT=k_sb[0:64, ks], rhs=q_sb[0:64, qs],
                    start=True, stop=True, tile_position=(0, 0))
                nc.tensor.matmul(
                    slab[:, 512:1024], lhsT=k_sb[64:128, ks], rhs=q_sb[64:128, qs],
                    start=True, stop=True, tile_position=(64, 0))
                return slab

            # ---- deferred-emission machinery ----
            # norm state per block b = it*4+hp
            st = {}

            def av_pair(b, jt):
                """AV accumulation for block b's exps at j-tile jt."""
                hp = b % 4
                if jt == 0:
                    a0 = avp.tile([128, 512], F32, tag="av0")
                    a1 = avp.tile([128, 512], F32, tag="av1")
                    st[b]["av"] = (a0, a1)
                av0, av1 = st[b]["av"]
                exps = st[b]["exps"][jt]
                nc.tensor.matmul(
                    av0[0:65, :],
                    lhsT=vt_sb[:, jt * 520 + (2 * hp) * 65: jt * 520 + (2 * hp) * 65 + 65],
                    rhs=exps[:, 0:512],
                    start=(jt == 0), stop=(jt == JT - 1))
                nc.tensor.matmul(
                    av1[0:65, :],
                    lhsT=vt_sb[:, jt * 520 + (2 * hp + 1) * 65: jt * 520 + (2 * hp + 1) * 65 + 65],
                    rhs=exps[:, 512:1024],
                    start=(jt == 0), stop=(jt == JT - 1))

            def norm_start(b):
                """After AV(b,7): drain AV banks, gather den rows (SBUF->SBUF)."""
                av0, av1 = st[b]["av"]
                avr = avrp.tile([65, 1024], F32, tag="avr")
                nc.vector.tensor_copy(avr[:, 0:512], av0[0:65, :])
                nc.vector.tensor_copy(avr[:, 512:1024], av1[0:65, :])
                stile = normp.tile([128, 8], F32, tag="stile")
                # b15's chain rides the scalar HWDGE ring so the tail chains
                # don't head-of-line block each other on the sync ring
                eng = nc.scalar if b >= 15 else nc.sync
                eng.dma_start(out=stile, in_=avr[64:65, :])
                st[b]["avr"] = avr
                st[b]["stile"] = stile

            def norm_recip(b):
                """reciprocal + scatter to DRAM + stride-0 broadcast reads."""
                it, hp = divmod(b, 4)
                stile_r = normp.tile([128, 8], F32, tag="stile_r")
                nc.vector.reciprocal(stile_r, st[b]["stile"])
                eng = nc.scalar if b >= 15 else nc.sync
                eng.dma_start(
                    out=sscratch2[it, hp].rearrange("(p f) -> p f", p=128),
                    in_=stile_r)
                sbc_a = normp.tile([64, 512], F32, tag="sbc_a")
                sbc_b = normp.tile([64, 512], F32, tag="sbc_b")
                eng.dma_start(
                    out=sbc_a,
                    in_=bass.AP(tensor=sscratch2, offset=b * 1024,
                                ap=[[0, 64], [1, 512]]))
                eng.dma_start(
                    out=sbc_b,
                    in_=bass.AP(tensor=sscratch2, offset=b * 1024 + 512,
                                ap=[[0, 64], [1, 512]]))
                st[b]["sbc"] = (sbc_a, sbc_b)

            def norm_mul(b):
                """normalized AV in f16."""
                avr = st[b]["avr"]
                sbc_a, sbc_b = st[b]["sbc"]
                avn = avnp.tile([128, 512], F16, tag="avn")
                nc.vector.tensor_mul(avn[0:64, :], avr[0:64, 0:512], sbc_a)
                nc.vector.tensor_mul(avn[64:128, :], avr[0:64, 512:1024], sbc_b)
                st[b]["avn"] = avn

            def oproj_mm(b):
                """ob0 (and for it3: ob1) matmul for cc=hp of block b's it."""
                it, hp = divmod(b, 4)
                if hp == 0:
                    y0_tile = yp.tile([128, 512], F32, tag="yps")
                    st[("y0", it)] = y0_tile
                    if it == 3:
                        y1_tile = pp.tile([128, 512], F32, tag="pps")
                        st[("y1", it)] = y1_tile
                nc.tensor.matmul(
                    st[("y0", it)],
                    lhsT=wot[:, hp * OC + 0 * 128: hp * OC + 1 * 128],
                    rhs=st[b]["avn"],
                    start=(hp == 0), stop=(hp == 3))
                if it == 3:
                    nc.tensor.matmul(
                        st[("y1", it)],
                        lhsT=wot[:, hp * OC + 1 * 128: hp * OC + 2 * 128],
                        rhs=st[b]["avn"],
                        start=(hp == 0), stop=(hp == 3))

            def oproj_fin(it):
                """drain ob0, run+drain ob1, store y."""
                yps = st[("y0", it)]
                ysb = yop.tile([128, 512], F32, tag="ysb")
                nc.vector.tensor_scalar_add(ysb, yps, bo_col[:, 0:1])
                seng = nc.scalar if it >= 2 else nc.sync
                seng.dma_start(
                    out=y[0:128, it * 512:(it + 1) * 512], in_=ysb)
                if it == 3:
                    yps1 = st[("y1", it)]
                else:
                    yps1 = yp.tile([128, 512], F32, tag="yps")
                    for cc in range(4):
                        nc.tensor.matmul(
                            yps1,
                            lhsT=wot[:, cc * OC + 1 * 128: cc * OC + 2 * 128],
                            rhs=st[it * 4 + cc]["avn"],
                            start=(cc == 0), stop=(cc == 3))
                ysb1 = yop.tile([128, 512], F32, tag="ysb")
                nc.vector.tensor_scalar_add(ysb1, yps1, bo_col[:, 1:2])
                seng.dma_start(
                    out=y[128:256, it * 512:(it + 1) * 512], in_=ysb1)

            # pending actions: (target_block, fn)
            pend = []

            def flush(b):
                keep = []
                for tb, fn in pend:
                    if tb <= b:
                        fn()
                    else:
                        keep.append((tb, fn))
                pend[:] = keep

            # per-block projection schedule: {block: {jt: [emitter, ...]}}
            # block 0 rotates through the idle av0/av1/pp banks; blocks 1-2
            # use pp+yp (yp is idle until block 4); later blocks only carry
            # one qproj via pp per block (ring reuse 8 steps apart = no
            # drain bubble on the PE stream).
            _rot["seq"] = [(avp, "av0"), (avp, "av1"), (pp, "pps")] * 3 + \
                          [(pp, "pps"), (yp, "yps")] * 3 + [(pp, "pps")] * 40
            proj_sched = {
                0: {0: [lambda: kproj_half(1, 0)], 1: [lambda: kproj_half(1, 1)],
                    2: [lambda: qproj(1, 0)],
                    3: [lambda: vproj(1), lambda: vproj(2)],
                    4: [lambda: vproj(3), lambda: vproj(4)],
                    5: [lambda: vproj(5), lambda: vproj(6)],
                    6: [lambda: vproj(7)]},
                1: {2: [lambda: kproj_half(2, 0)], 3: [lambda: kproj_half(2, 1)],
                    5: [lambda: qproj(2, 0)]},
                2: {2: [lambda: kproj_half(3, 0)], 3: [lambda: kproj_half(3, 1)],
                    5: [lambda: qproj(3, 0)]},
            }
            # lazy qproj for block b+1 at step 4 of block b (b >= 3)
            for _b in range(3, 15):
                _it1, _hp1 = divmod(_b + 1, 4)
                proj_sched.setdefault(_b, {}).setdefault(4, []).append(
                    (lambda h, i: lambda: qproj(h, i))(_hp1, _it1))

            # ---- prologue projections (through the idle slab ring) ----
            pk2 = slabp.tile([128, 1024], F32, tag="slab")
            for ntile in range(2):
                for ec in range(4):
                    nc.tensor.matmul(
                        pk2[:, ntile * 512:(ntile + 1) * 512],
                        lhsT=wkt[:, ec * 512: ec * 512 + 128],
                        rhs=ctx_sb[:, ec * NK + ntile * 512: ec * NK + (ntile + 1) * 512],
                        start=(ec == 0), stop=(ec == 3))
                nc.vector.tensor_copy(
                    k_sb[:, ntile * 512:(ntile + 1) * 512],
                    pk2[:, ntile * 512:(ntile + 1) * 512])
            pqv = slabp.tile([128, 1024], F32, tag="slab")
            for ec in range(2):
                nc.tensor.matmul(
                    pqv[:, 0:512],
                    lhsT=wqt[:, ec * 512: ec * 512 + 128],
                    rhs=x_sb[:, ec * NQ: ec * NQ + 512],
                    start=(ec == 0), stop=(ec == 1))
            for ec in range(4):
                nc.tensor.matmul(
                    pqv[:, 512:1024],
                    lhsT=ctx_sb[:, ec * NK: ec * NK + 128],
                    rhs=wvt[:, ec * 512:(ec + 1) * 512],
                    start=(ec == 0), stop=(ec == 3))
            nc.vector.tensor_copy(q_sb[:, 0:512], pqv[:, 0:512])
            vt_t0 = vt_sb[:, 0:520].rearrange("p (h c) -> p h c", h=H)[:, :, 0:64]
            nc.vector.tensor_copy(vt_t0, pqv[:, 512:1024].rearrange("p (h c) -> p h c", c=64))

            # ---- main loop over blocks b = it*4 + hp ----
            # AVs for block b run during block b+1 (deferred), so projection
            # bubbles and AV-bank drains never stall the sim->exp stream.
            for b in range(16):
                it, hp = divmod(b, 4)
                st[b] = {"exps": []}
                flush(b)          # oproj MMs / oproj_fin due at this block

                slab = sim_emit(hp, it, 0)
                for jt in range(JT):
                    nslab = sim_emit(hp, it, jt + 1) if jt + 1 < JT else None
                    exps = ep.tile([128, 1024], F16, tag="exps")
                    nc.scalar.activation(exps, slab, EXP, bias=0.0, scale=SCALE)
                    st[b]["exps"].append(exps)
                    for fn in proj_sched.get(b, {}).get(jt, []):
                        fn()
                    if jt == 3:
                        # deferred DVE work, dependency-free by now
                        if b >= 2:
                            norm_recip(b - 2)
                        if b >= 3:
                            norm_mul(b - 3)
                    if b >= 1:
                        av_pair(b - 1, jt)
                    slab = nslab
                if b >= 1:
                    norm_start(b - 1)
                pend.append((b + 4, (lambda bb: lambda: oproj_mm(bb))(b)))
                if hp == 3:
                    pend.append((b + 5, (lambda ii: lambda: oproj_fin(ii))(it)))

            # block 15's own AVs: paced by the trailing exps on ACT
            for jt in range(JT):
                av_pair(15, jt)
            norm_start(15)

            # ---- tail: remaining recip/mul/oproj in dependency order ----
            # recip(15) before muls so the DVE in-order queue doesn't hold
            # the b15 chain behind b14's broadcast round-trip
            norm_recip(14)
            norm_recip(15)
            norm_mul(13)
            flush(16)            # fin(2), oproj_mm(12)
            norm_mul(14)
            flush(17)            # oproj_mm(13)
            norm_mul(15)
            flush(99)            # oproj_mm(14), oproj_mm(15), fin(3)

    _split_excess_waits(nc)
    return nc


_CACHED = None


def kernel(x, context, Wq, Wk, Wv, Wo, bo):
    global _CACHED
    if _CACHED is None:
        _CACHED = _build()
    nc = _CACHED

    x = np.asarray(x, dtype=np.float32)
    context = np.asarray(context, dtype=np.float32)
    xf = x.reshape(B, EQ, 64 * 64)
    cf = context.reshape(B, EK, 32 * 32)
    WqT = np.ascontiguousarray(np.asarray(Wq, np.float32).T.astype(np.float16))
    WkT = np.ascontiguousarray(np.asarray(Wk, np.float32).T.astype(np.float16))
    WvT = np.ascontiguousarray(np.asarray(Wv, np.float32).T.astype(np.float16))
    WoT = np.ascontiguousarray(np.asarray(Wo, np.float32).T.astype(np.float16))
    bo = np.ascontiguousarray(np.asarray(bo, np.float32))

    in_maps = []
    for core in range(8):
        b, half = core // 2, core % 2
        in_maps.append({
            "x_s": np.ascontiguousarray(xf[b, :, half * NQ:(half + 1) * NQ]).astype(np.float16),
            "ctx_s": np.ascontiguousarray(cf[b]).astype(np.float16),
            "WqT": WqT, "WkT": WkT, "WvT": WvT, "WoT": WoT, "bo": bo,
        })

    res = run_bass_kernel_spmd(nc, in_maps, list(range(8)))
    kernel.last_results = res

    out = np.empty((B, OC, 64 * 64), dtype=np.float32)
    for core in range(8):
        b, half = core // 2, core % 2
        out[b, :, half * NQ:(half + 1) * NQ] = res.results[core]["y"]
    return out.reshape(B, OC, 64, 64)


# revision 23
# speedup vs baseline: 1.0276x; 1.0276x over previous
"""CrossAttention Trainium2 kernel (8 NeuronCores, SPMD).

Problem: x [4,256,64,64], context [4,512,32,32], 8 heads x 64 dim,
q = Wq@x, k = Wk@ctx, v = Wv@ctx, attn = softmax(q^T k / 8), out = Wo@(v attn^T) + bo.

Sharding: fully data-parallel over (batch, query-spatial-half) -> 8 shards.
Each core computes K/V for its batch (duplicated per pair) and attention +
output projection for its 2048 query positions. Zero collectives.

Pipeline (ScalarE exp is the roofline engine at ~110us/core; everything is
scheduled so ACT never stalls):
  - dummy PE warm-up burst during the input DMA so HAM unthrottles before the
    first projection;
  - first exp issues after only Kproj(hp0)+Qproj(hp0,it0)+1 sim (~7us); all
    other K/V/Q projections stream through a 1-bank PSUM pool inside it0's
    PE slack;
  - simT[j,i] = k^T q per head-pair (two K=64 matmuls packed via
    tile_position); exp on ScalarE out of PSUM; AV accumulation with M=65
    (ones column = softmax denominator);
  - per-(it,hp) normalization is a 4-stage pipeline (DVE drain -> DRAM
    bounce -> reciprocal -> stride-0 partition-broadcast -> multiply) whose
    stages are emitted 1-2 blocks later in the DVE stream so no DVE
    instruction ever waits in-queue;
  - the output projection for block b is emitted at block b+3 (ob-major,
    single PSUM bank), so the kernel tail is just one normalization chain.
"""
import os
import sys
import numpy as np

for _p in ("/opt/trn_rl_repo", "/root/.axon_site/_ro/trn_rl_repo"):
    if os.path.isdir(_p) and _p not in sys.path:
        sys.path.insert(0, _p)

import concourse.bass as bass
import concourse.mybir as mybir
from concourse.tile import TileContext
from concourse.bass_utils import run_bass_kernel_spmd

F32 = mybir.dt.float32
F16 = mybir.dt.float16
EXP = mybir.ActivationFunctionType.Exp

B, H, D = 4, 8, 64
EQ, EK = 256, 512          # x channels, ctx channels
NQ, NK = 2048, 1024        # per-core query positions, kv positions
OC = 256                   # output channels
SCALE = D ** -0.5
IT, JT = NQ // 512, NK // 128   # 4 i-tiles of 512, 8 j-tiles of 128


def _split_excess_waits(nc, max_waits=1):
    """This walrus build rejects instructions carrying >max_waits sem waits;
    move the extras onto standalone nops just before (same engine, in-order,
    so semantics are unchanged)."""
    n_new = 0
    for f in nc.m.functions:
        for bb in f.blocks:
            insts = list(bb.instructions)
            out = []
            changed = False
            for inst in insts:
                si = inst.sync_info
                if si is not None and si.on_wait and len(si.on_wait) > max_waits:
                    waits = list(si.on_wait)
                    for w in waits[:-max_waits]:
                        nop = mybir.InstNoOp(
                            name=f"I-splitw-{n_new}",
                            sync_info=mybir.SyncInfo(on_wait=[w], on_update=[]),
                        )
                        nop.engine = inst.engine
                        n_new += 1
                        out.append(nop)
                        nc.register_instruction(nop, overwrite=True)
                    si.on_wait = waits[-max_waits:]
                    inst.sync_info = si
                    changed = True
                out.append(inst)
            if changed:
                bb.instructions.clear()
                bb.instructions.extend(out)
    return n_new


def _build():
    nc = bass.Bass()
    x_s = nc.declare_dram_parameter("x_s", [EQ, NQ], F16, isOutput=False)
    ctx_s = nc.declare_dram_parameter("ctx_s", [EK, NK], F16, isOutput=False)
    WqT = nc.declare_dram_parameter("WqT", [EQ, 512], F16, isOutput=False)
    WkT = nc.declare_dram_parameter("WkT", [EK, 512], F16, isOutput=False)
    WvT = nc.declare_dram_parameter("WvT", [EK, 512], F16, isOutput=False)
    WoT = nc.declare_dram_parameter("WoT", [512, OC], F16, isOutput=False)
    bo = nc.declare_dram_parameter("bo", [OC], F32, isOutput=False)
    y = nc.declare_dram_parameter("y", [OC, NQ], F32, isOutput=True)

    sscratch2 = nc.dram_tensor("sscratch2", [IT, 4, 1024], F32)

    with TileContext(nc) as tc:
        with (
            tc.tile_pool(name="consts", bufs=1) as cp,
            tc.tile_pool(name="qkv", bufs=1) as qp,
            tc.tile_pool(name="exps", bufs=16) as ep,
            tc.tile_pool(name="avrp", bufs=4) as avrp,
            tc.tile_pool(name="avnp", bufs=8) as avnp,
            tc.tile_pool(name="normp", bufs=3) as normp,
            tc.tile_pool(name="yout", bufs=2) as yop,
            tc.tile_pool(name="slab", bufs=2, space="PSUM") as slabp,
            tc.tile_pool(name="avp", bufs=1, space="PSUM") as avp,
            tc.tile_pool(name="yp", bufs=1, space="PSUM") as yp,
            tc.tile_pool(name="pp", bufs=1, space="PSUM") as pp,
        ):
            # ---- warm-up source (no DMA dep; memset only) ----
            warm_src = cp.tile([128, 512], F16, tag="warm_src")
            nc.vector.memset(warm_src, 0.25)

            # ---- DMA loads: critical-path-first order ----
            # exp0 needs only ctx + wkt[hp0] + wqt[hp0] + x[:, it0];
            # everything else streams in behind it.
            wkt = cp.tile([128, 4 * 512], F16, tag="wkt")
            wvt = cp.tile([128, 4 * 512], F16, tag="wvt")
            ctx_sb = cp.tile([128, 4 * NK], F16, tag="ctx_sb")
            wqt = cp.tile([128, 2 * 512], F16, tag="wqt")
            x_sb = cp.tile([128, 2 * NQ], F16, tag="x_sb")
            wot = cp.tile([128, 4 * OC], F16, tag="wot")
            bo_col = cp.tile([128, 2], F32, tag="bo_col")
            # Loads split between the sync HWDGE ring and the GpSimd SWDGE
            # ring — each ring is one in-order queue, so one ring would
            # serialize the whole 3.7MB load. ACT's ring stays clean so the
            # exp table load isn't delayed behind load triggers.
            # sync ring: ctx ec0/ec1 -> exp0 critical path -> rest
            nc.sync.dma_start(out=ctx_sb[:, 0:NK], in_=ctx_s[0:128, :])
            nc.sync.dma_start(out=ctx_sb[:, NK:2 * NK], in_=ctx_s[128:256, :])
            for ec in range(4):
                nc.sync.dma_start(out=wkt[:, ec * 512:ec * 512 + 128],
                                  in_=WkT[ec * 128:(ec + 1) * 128, 0:128])
            for ec in range(2):
                nc.sync.dma_start(out=wqt[:, ec * 512:ec * 512 + 128],
                                  in_=WqT[ec * 128:(ec + 1) * 128, 0:128])
                nc.sync.dma_start(out=x_sb[:, ec * NQ:ec * NQ + 512],
                                  in_=x_s[ec * 128:(ec + 1) * 128, 0:512])
            for ec in range(4):
                nc.sync.dma_start(out=wkt[:, ec * 512 + 128:(ec + 1) * 512],
                                  in_=WkT[ec * 128:(ec + 1) * 128, 128:512])
            for ec in range(2):
                nc.sync.dma_start(out=x_sb[:, ec * NQ + 512:(ec + 1) * NQ],
                                  in_=x_s[ec * 128:(ec + 1) * 128, 512:NQ])
            for ec in range(4):
                nc.sync.dma_start(out=wot[:, ec * OC:(ec + 1) * OC],
                                  in_=WoT[ec * 128:(ec + 1) * 128, :])
            for ob in range(2):
                nc.sync.dma_start(out=bo_col[:, ob:ob + 1],
                                  in_=bo[ob * 128:(ob + 1) * 128])
            # gpsimd ring: ctx ec2/ec3, then wvt, then wqt rest
            nc.gpsimd.dma_start(out=ctx_sb[:, 2 * NK:3 * NK], in_=ctx_s[256:384, :])
            nc.gpsimd.dma_start(out=ctx_sb[:, 3 * NK:4 * NK], in_=ctx_s[384:512, :])
            for ec in range(4):
                nc.gpsimd.dma_start(out=wvt[:, ec * 512:(ec + 1) * 512],
                                    in_=WvT[ec * 128:(ec + 1) * 128, :])
            for ec in range(2):
                nc.gpsimd.dma_start(out=wqt[:, ec * 512 + 128:(ec + 1) * 512],
                                    in_=WqT[ec * 128:(ec + 1) * 128, 128:512])

            # persistent activations
            q_sb = qp.tile([128, 4 * NQ], F16, tag="q_sb")      # [hp, i]
            k_sb = qp.tile([128, 4 * NK], F16, tag="k_sb")      # [hp, j]
            vt_sb = qp.tile([128, JT * 520], F16, tag="vt_sb")  # [jt, h*65 + c]

            # ones columns of vt (col 64 of each 65-block)
            vt_4d = vt_sb.rearrange("p (j h c) -> p j h c", j=JT, h=H)
            ones_f32 = cp.tile([128, JT * H], F32, tag="ones_f32")
            nc.vector.memset(ones_f32, 1.0)
            nc.vector.tensor_copy(
                vt_4d[:, :, :, 64:65],
                ones_f32.rearrange("p (j h) -> p j h", j=JT).unsqueeze(-1))

            # ---- PE warm-up: dummy matmuls during the DMA wait ----
            # long enough to bridge into kproj0 so HAM stays at K=8/8
            for _ in range(10):
                wps = pp.tile([128, 512], F32, tag="pps")
                nc.tensor.matmul(wps, lhsT=warm_src[:, 0:128],
                                 rhs=warm_src, start=True, stop=True)

            # ---- projection emitters ----
            # rotate projection PSUM tiles through idle banks so the
            # single-buffer drain bubble never blocks the PE stream
            _rot = {"seq": []}

            def _ptile():
                if not _rot["seq"]:
                    _rot["seq"] = [(pp, "pps")]
                pool, tag = _rot["seq"].pop(0)
                t = pool.tile([128, 512], F32, tag=tag)
                return t

            def kproj_half(hp, ntile):
                pk = _ptile()
                for ec in range(4):
                    nc.tensor.matmul(
                        pk,
                        lhsT=wkt[:, ec * 512 + hp * 128: ec * 512 + (hp + 1) * 128],
                        rhs=ctx_sb[:, ec * NK + ntile * 512: ec * NK + (ntile + 1) * 512],
                        start=(ec == 0), stop=(ec == 3))
                nc.vector.tensor_copy(
                    k_sb[:, hp * NK + ntile * 512: hp * NK + (ntile + 1) * 512], pk)

            def vproj(jt):
                pv = _ptile()
                for ec in range(4):
                    nc.tensor.matmul(
                        pv,
                        lhsT=ctx_sb[:, ec * NK + jt * 128: ec * NK + (jt + 1) * 128],
                        rhs=wvt[:, ec * 512:(ec + 1) * 512],
                        start=(ec == 0), stop=(ec == 3))
                vt_t = vt_sb[:, jt * 520:(jt + 1) * 520].rearrange(
                    "p (h c) -> p h c", h=H)[:, :, 0:64]
                nc.vector.tensor_copy(vt_t, pv.rearrange("p (h c) -> p h c", c=64))

            def qproj(hp, it):
                pq = _ptile()
                for ec in range(2):
                    nc.tensor.matmul(
                        pq,
                        lhsT=wqt[:, ec * 512 + hp * 128: ec * 512 + (hp + 1) * 128],
                        rhs=x_sb[:, ec * NQ + it * 512: ec * NQ + (it + 1) * 512],
                        start=(ec == 0), stop=(ec == 1))
                nc.vector.tensor_copy(
                    q_sb[:, hp * NQ + it * 512: hp * NQ + (it + 1) * 512], pq)

            def sim_emit(hp, it, jt):
                slab = slabp.tile([128, 1024], F32, tag="slab")
                ks = slice(hp * NK + jt * 128, hp * NK + (jt + 1) * 128)
                qs = slice(hp * NQ + it * 512, hp * NQ + (it + 1) * 512)
                nc.tensor.matmul(
                    slab[:, 0:512], lhsT=k_sb[0:64, ks], rhs=q_sb[0:64, qs],
                    start=True, stop=True, tile_position=(0, 0))
                nc.tensor.matmul(
                    slab[:, 512:1024], lhsT=k_sb[64:128, ks], rhs=q_sb[64:128, qs],
                    start=True, stop=True, tile_position=(64, 0))
                return slab

            # ---- deferred-emission machinery ----
            # norm state per block b = it*4+hp
            st = {}

            def av_pair(b, jt):
                """AV accumulation for block b's exps at j-tile jt."""
                hp = b % 4
                if jt == 0:
                    a0 = avp.tile([128, 512], F32, tag="av0")
                    a1 = avp.tile([128, 512], F32, tag="av1")
                    st[b]["av"] = (a0, a1)
                av0, av1 = st[b]["av"]
                exps = st[b]["exps"][jt]
                nc.tensor.matmul(
                    av0[0:65, :],
                    lhsT=vt_sb[:, jt * 520 + (2 * hp) * 65: jt * 520 + (2 * hp) * 65 + 65],
                    rhs=exps[:, 0:512],
                    start=(jt == 0), stop=(jt == JT - 1))
                nc.tensor.matmul(
                    av1[0:65, :],
                    lhsT=vt_sb[:, jt * 520 + (2 * hp + 1) * 65: jt * 520 + (2 * hp + 1) * 65 + 65],
                    rhs=exps[:, 512:1024],
                    start=(jt == 0), stop=(jt == JT - 1))

            def norm_start(b):
                """After AV(b,7): drain AV banks, gather den rows (SBUF->SBUF)."""
                av0, av1 = st[b]["av"]
                avr = avrp.tile([65, 1024], F32, tag="avr")
                nc.vector.tensor_copy(avr[:, 0:512], av0[0:65, :])
                nc.vector.tensor_copy(avr[:, 512:1024], av1[0:65, :])
                stile = normp.tile([128, 8], F32, tag="stile")
                # b15's chain rides the scalar HWDGE ring so the tail chains
                # don't head-of-line block each other on the sync ring
                eng = nc.scalar if b >= 15 else nc.sync
                eng.dma_start(out=stile, in_=avr[64:65, :])
                st[b]["avr"] = avr
                st[b]["stile"] = stile

            def norm_recip(b):
                """reciprocal + scatter to DRAM + stride-0 broadcast reads."""
                it, hp = divmod(b, 4)
                stile_r = normp.tile([128, 8], F32, tag="stile_r")
                nc.vector.reciprocal(stile_r, st[b]["stile"])
                eng = nc.scalar if b >= 15 else nc.sync
                eng.dma_start(
                    out=sscratch2[it, hp].rearrange("(p f) -> p f", p=128),
                    in_=stile_r)
                sbc_a = normp.tile([64, 512], F32, tag="sbc_a")
                sbc_b = normp.tile([64, 512], F32, tag="sbc_b")
                eng.dma_start(
                    out=sbc_a,
                    in_=bass.AP(tensor=sscratch2, offset=b * 1024,
                                ap=[[0, 64], [1, 512]]))
                eng.dma_start(
                    out=sbc_b,
                    in_=bass.AP(tensor=sscratch2, offset=b * 1024 + 512,
                                ap=[[0, 64], [1, 512]]))
                st[b]["sbc"] = (sbc_a, sbc_b)

            def norm_mul(b):
                """normalized AV in f16."""
                avr = st[b]["avr"]
                sbc_a, sbc_b = st[b]["sbc"]
                avn = avnp.tile([128, 512], F16, tag="avn")
                nc.vector.tensor_mul(avn[0:64, :], avr[0:64, 0:512], sbc_a)
                nc.vector.tensor_mul(avn[64:128, :], avr[0:64, 512:1024], sbc_b)
                st[b]["avn"] = avn

            def oproj_mm(b):
                """ob0 (and for it3: ob1) matmul for cc=hp of block b's it."""
                it, hp = divmod(b, 4)
                if hp == 0:
                    y0_tile = yp.tile([128, 512], F32, tag="yps")
                    st[("y0", it)] = y0_tile
                    if it == 3:
                        y1_tile = pp.tile([128, 512], F32, tag="pps")
                        st[("y1", it)] = y1_tile
                nc.tensor.matmul(
                    st[("y0", it)],
                    lhsT=wot[:, hp * OC + 0 * 128: hp * OC + 1 * 128],
                    rhs=st[b]["avn"],
                    start=(hp == 0), stop=(hp == 3))
                if it == 3:
                    nc.tensor.matmul(
                        st[("y1", it)],
                        lhsT=wot[:, hp * OC + 1 * 128: hp * OC + 2 * 128],
                        rhs=st[b]["avn"],
                        start=(hp == 0), stop=(hp == 3))

            def oproj_fin(it):
                """drain ob0, run+drain ob1, store y."""
                yps = st[("y0", it)]
                ysb = yop.tile([128, 512], F32, tag="ysb")
                nc.vector.tensor_scalar_add(ysb, yps, bo_col[:, 0:1])
                seng = nc.scalar if it >= 2 else nc.sync
                seng.dma_start(
                    out=y[0:128, it * 512:(it + 1) * 512], in_=ysb)
                if it == 3:
                    yps1 = st[("y1", it)]
                else:
                    yps1 = yp.tile([128, 512], F32, tag="yps")
                    for cc in range(4):
                        nc.tensor.matmul(
                            yps1,
                            lhsT=wot[:, cc * OC + 1 * 128: cc * OC + 2 * 128],
                            rhs=st[it * 4 + cc]["avn"],
                            start=(cc == 0), stop=(cc == 3))
                ysb1 = yop.tile([128, 512], F32, tag="ysb")
                nc.vector.tensor_scalar_add(ysb1, yps1, bo_col[:, 1:2])
                seng.dma_start(
                    out=y[128:256, it * 512:(it + 1) * 512], in_=ysb1)

            # pending actions: (target_block, fn)
            pend = []

            def flush(b):
                keep = []
                for tb, fn in pend:
                    if tb <= b:
                        fn()
                    else:
                        keep.append((tb, fn))
                pend[:] = keep

            # per-block projection schedule: {block: {jt: [emitter, ...]}}
            # block 0 rotates through the idle av0/av1/pp banks; blocks 1-2
            # use pp+yp (yp is idle until block 4); later blocks only carry
            # one qproj via pp per block (ring reuse 8 steps apart = no
            # drain bubble on the PE stream).
            _rot["seq"] = [(avp, "av0"), (avp, "av1"), (pp, "pps")] * 3 + \
                          [(pp, "pps"), (yp, "yps")] * 3 + [(pp, "pps")] * 40
            proj_sched = {
                0: {0: [lambda: kproj_half(1, 0)], 1: [lambda: kproj_half(1, 1)],
                    2: [lambda: qproj(1, 0)],
                    3: [lambda: vproj(1), lambda: vproj(2)],
                    4: [lambda: vproj(3), lambda: vproj(4)],
                    5: [lambda: vproj(5), lambda: vproj(6)],
                    6: [lambda: vproj(7)]},
                1: {2: [lambda: kproj_half(2, 0)], 3: [lambda: kproj_half(2, 1)],
                    5: [lambda: qproj(2, 0)]},
                2: {2: [lambda: kproj_half(3, 0)], 3: [lambda: kproj_half(3, 1)],
                    5: [lambda: qproj(3, 0)]},
            }
            # lazy qproj for block b+1 at step 4 of block b (b >= 3)
            for _b in range(3, 15):
                _it1, _hp1 = divmod(_b + 1, 4)
                proj_sched.setdefault(_b, {}).setdefault(4, []).append(
                    (lambda h, i: lambda: qproj(h, i))(_hp1, _it1))

            # ---- prologue projections (through the idle slab ring) ----
            pk2 = slabp.tile([128, 1024], F32, tag="slab")
            for ntile in range(2):
                for ec in range(4):
                    nc.tensor.matmul(
                        pk2[:, ntile * 512:(ntile + 1) * 512],
                        lhsT=wkt[:, ec * 512: ec * 512 + 128],
                        rhs=ctx_sb[:, ec * NK + ntile * 512: ec * NK + (ntile + 1) * 512],
                        start=(ec == 0), stop=(ec == 3))
                nc.vector.tensor_copy(
                    k_sb[:, ntile * 512:(ntile + 1) * 512],
                    pk2[:, ntile * 512:(ntile + 1) * 512])
            pqv = slabp.tile([128, 1024], F32, tag="slab")
            for ec in range(2):
                nc.tensor.matmul(
                    pqv[:, 0:512],
                    lhsT=wqt[:, ec * 512: ec * 512 + 128],
                    rhs=x_sb[:, ec * NQ: ec * NQ + 512],
                    start=(ec == 0), stop=(ec == 1))
            for ec in range(4):
                nc.tensor.matmul(
                    pqv[:, 512:1024],
                    lhsT=ctx_sb[:, ec * NK: ec * NK + 128],
                    rhs=wvt[:, ec * 512:(ec + 1) * 512],
                    start=(ec == 0), stop=(ec == 3))
            nc.vector.tensor_copy(q_sb[:, 0:512], pqv[:, 0:512])
            vt_t0 = vt_sb[:, 0:520].rearrange("p (h c) -> p h c", h=H)[:, :, 0:64]
            nc.vector.tensor_copy(vt_t0, pqv[:, 512:1024].rearrange("p (h c) -> p h c", c=64))

            # ---- main loop over blocks b = it*4 + hp ----
            # AVs for block b run during block b+1 (deferred), so projection
            # bubbles and AV-bank drains never stall the sim->exp stream.
            for b in range(16):
                it, hp = divmod(b, 4)
                st[b] = {"exps": []}
                flush(b)          # oproj MMs / oproj_fin due at this block

                slab = sim_emit(hp, it, 0)
                for jt in range(JT):
                    nslab = sim_emit(hp, it, jt + 1) if jt + 1 < JT else None
                    exps = ep.tile([128, 1024], F16, tag="exps")
                    nc.scalar.activation(exps, slab, EXP, bias=0.0, scale=SCALE)
                    st[b]["exps"].append(exps)
                    for fn in proj_sched.get(b, {}).get(jt, []):
                        fn()
                    if jt == 3:
                        # deferred DVE work, dependency-free by now
                        if b >= 2:
                            norm_recip(b - 2)
                        if b >= 3:
                            norm_mul(b - 3)
                    if b >= 1:
                        av_pair(b - 1, jt)
                    slab = nslab
                if b >= 1:
                    norm_start(b - 1)
                pend.append((b + 4, (lambda bb: lambda: oproj_mm(bb))(b)))
                if hp == 3:
                    pend.append((b + 5, (lambda ii: lambda: oproj_fin(ii))(it)))

            # block 15's own AVs: paced by the trailing exps on ACT
            for jt in range(JT):
                av_pair(15, jt)
            norm_start(15)

            # ---- tail: remaining recip/mul/oproj in dependency order ----
            # recip(15) before muls so the DVE in-order queue doesn't hold
            # the b15 chain behind b14's broadcast round-trip
            norm_recip(14)
            norm_recip(15)
            norm_mul(13)
            flush(16)            # fin(2), oproj_mm(12)
            norm_mul(14)
            flush(17)            # oproj_mm(13)
            norm_mul(15)
            flush(99)            # oproj_mm(14), oproj_mm(15), fin(3)

    _split_excess_waits(nc)
    return nc


_CACHED = None


def kernel(x, context, Wq, Wk, Wv, Wo, bo):
    global _CACHED
    if _CACHED is None:
        _CACHED = _build()
    nc = _CACHED

    x = np.asarray(x, dtype=np.float32)
    context = np.asarray(context, dtype=np.float32)
    xf = x.reshape(B, EQ, 64 * 64)
    cf = context.reshape(B, EK, 32 * 32)
    WqT = np.ascontiguousarray(np.asarray(Wq, np.float32).T.astype(np.float16))
    WkT = np.ascontiguousarray(np.asarray(Wk, np.float32).T.astype(np.float16))
    WvT = np.ascontiguousarray(np.asarray(Wv, np.float32).T.astype(np.float16))
    WoT = np.ascontiguousarray(np.asarray(Wo, np.float32).T.astype(np.float16))
    bo = np.ascontiguousarray(np.asarray(bo, np.float32))

    in_maps = []
    for core in range(8):
        b, half = core // 2, core % 2
        in_maps.append({
            "x_s": np.ascontiguousarray(xf[b, :, half * NQ:(half + 1) * NQ]).astype(np.float16),
            "ctx_s": np.ascontiguousarray(cf[b]).astype(np.float16),
            "WqT": WqT, "WkT": WkT, "WvT": WvT, "WoT": WoT, "bo": bo,
        })

    res = run_bass_kernel_spmd(nc, in_maps, list(range(8)))
    kernel.last_results = res

    out = np.empty((B, OC, 64 * 64), dtype=np.float32)
    for core in range(8):
        b, half = core // 2, core % 2
        out[b, :, half * NQ:(half + 1) * NQ] = res.results[core]["y"]
    return out.reshape(B, OC, 64, 64)


# revision 27
# speedup vs baseline: 1.0289x; 1.0012x over previous
"""CrossAttention Trainium2 kernel (8 NeuronCores, SPMD).

Problem: x [4,256,64,64], context [4,512,32,32], 8 heads x 64 dim,
q = Wq@x, k = Wk@ctx, v = Wv@ctx, attn = softmax(q^T k / 8), out = Wo@(v attn^T) + bo.

Sharding: fully data-parallel over (batch, query-spatial-half) -> 8 shards.
Each core computes K/V for its batch (duplicated per pair) and attention +
output projection for its 2048 query positions. Zero collectives.

Pipeline (ScalarE exp is the roofline engine at ~110us/core; everything is
scheduled so ACT never stalls):
  - dummy PE warm-up burst during the input DMA so HAM unthrottles before the
    first projection;
  - first exp issues after only Kproj(hp0)+Qproj(hp0,it0)+1 sim (~7us); all
    other K/V/Q projections stream through a 1-bank PSUM pool inside it0's
    PE slack;
  - simT[j,i] = k^T q per head-pair (two K=64 matmuls packed via
    tile_position); exp on ScalarE out of PSUM; AV accumulation with M=65
    (ones column = softmax denominator);
  - per-(it,hp) normalization is a 4-stage pipeline (DVE drain -> DRAM
    bounce -> reciprocal -> stride-0 partition-broadcast -> multiply) whose
    stages are emitted 1-2 blocks later in the DVE stream so no DVE
    instruction ever waits in-queue;
  - the output projection for block b is emitted at block b+3 (ob-major,
    single PSUM bank), so the kernel tail is just one normalization chain.
"""
import os
import sys
import numpy as np

for _p in ("/opt/trn_rl_repo", "/root/.axon_site/_ro/trn_rl_repo"):
    if os.path.isdir(_p) and _p not in sys.path:
        sys.path.insert(0, _p)

import concourse.bass as bass
import concourse.mybir as mybir
from concourse.tile import TileContext
from concourse.bass_utils import run_bass_kernel_spmd

F32 = mybir.dt.float32
F16 = mybir.dt.float16
EXP = mybir.ActivationFunctionType.Exp

B, H, D = 4, 8, 64
EQ, EK = 256, 512          # x channels, ctx channels
NQ, NK = 2048, 1024        # per-core query positions, kv positions
OC = 256                   # output channels
SCALE = D ** -0.5
IT, JT = NQ // 512, NK // 128   # 4 i-tiles of 512, 8 j-tiles of 128


def _split_excess_waits(nc, max_waits=1):
    """This walrus build rejects instructions carrying >max_waits sem waits;
    move the extras onto standalone nops just before (same engine, in-order,
    so semantics are unchanged)."""
    n_new = 0
    for f in nc.m.functions:
        for bb in f.blocks:
            insts = list(bb.instructions)
            out = []
            changed = False
            for inst in insts:
                si = inst.sync_info
                if si is not None and si.on_wait and len(si.on_wait) > max_waits:
                    waits = list(si.on_wait)
                    for w in waits[:-max_waits]:
                        nop = mybir.InstNoOp(
                            name=f"I-splitw-{n_new}",
                            sync_info=mybir.SyncInfo(on_wait=[w], on_update=[]),
                        )
                        nop.engine = inst.engine
                        n_new += 1
                        out.append(nop)
                        nc.register_instruction(nop, overwrite=True)
                    si.on_wait = waits[-max_waits:]
                    inst.sync_info = si
                    changed = True
                out.append(inst)
            if changed:
                bb.instructions.clear()
                bb.instructions.extend(out)
    return n_new


def _build():
    nc = bass.Bass()
    x_s = nc.declare_dram_parameter("x_s", [EQ, NQ], F16, isOutput=False)
    ctx_s = nc.declare_dram_parameter("ctx_s", [EK, NK], F16, isOutput=False)
    WqT = nc.declare_dram_parameter("WqT", [EQ, 512], F16, isOutput=False)
    WkT = nc.declare_dram_parameter("WkT", [EK, 512], F16, isOutput=False)
    WvT = nc.declare_dram_parameter("WvT", [EK, 512], F16, isOutput=False)
    WoT = nc.declare_dram_parameter("WoT", [512, OC], F16, isOutput=False)
    bo = nc.declare_dram_parameter("bo", [OC], F32, isOutput=False)
    y = nc.declare_dram_parameter("y", [OC, NQ], F32, isOutput=True)

    sscratch2 = nc.dram_tensor("sscratch2", [IT, 4, 1024], F32)

    with TileContext(nc) as tc:
        with (
            tc.tile_pool(name="consts", bufs=1) as cp,
            tc.tile_pool(name="qkv", bufs=1) as qp,
            tc.tile_pool(name="exps", bufs=16) as ep,
            tc.tile_pool(name="avrp", bufs=4) as avrp,
            tc.tile_pool(name="avnp", bufs=8) as avnp,
            tc.tile_pool(name="normp", bufs=3) as normp,
            tc.tile_pool(name="yout", bufs=2) as yop,
            tc.tile_pool(name="slab", bufs=2, space="PSUM") as slabp,
            tc.tile_pool(name="avp", bufs=1, space="PSUM") as avp,
            tc.tile_pool(name="yp", bufs=1, space="PSUM") as yp,
            tc.tile_pool(name="pp", bufs=1, space="PSUM") as pp,
        ):
            # ---- warm-up source (no DMA dep; memset only) ----
            warm_src = cp.tile([128, 512], F16, tag="warm_src")
            nc.vector.memset(warm_src, 0.25)

            # ---- DMA loads: critical-path-first order ----
            # exp0 needs only ctx + wkt[hp0] + wqt[hp0] + x[:, it0];
            # everything else streams in behind it.
            wkt = cp.tile([128, 4 * 512], F16, tag="wkt")
            wvt = cp.tile([128, 4 * 512], F16, tag="wvt")
            ctx_sb = cp.tile([128, 4 * NK], F16, tag="ctx_sb")
            wqt = cp.tile([128, 2 * 512], F16, tag="wqt")
            x_sb = cp.tile([128, 2 * NQ], F16, tag="x_sb")
            wot = cp.tile([128, 4 * OC], F16, tag="wot")
            bo_col = cp.tile([128, 2], F32, tag="bo_col")
            # Loads split between the sync HWDGE ring and the GpSimd SWDGE
            # ring — each ring is one in-order queue, so one ring would
            # serialize the whole 3.7MB load. ACT's ring stays clean so the
            # exp table load isn't delayed behind load triggers.
            # sync ring: x/wqt first (they unblock qv0 during the ctx wait),
            # then ctx + wkt for kproj, then the rest
            for ec in range(2):
                nc.sync.dma_start(out=wqt[:, ec * 512:ec * 512 + 128],
                                  in_=WqT[ec * 128:(ec + 1) * 128, 0:128])
                nc.sync.dma_start(out=x_sb[:, ec * NQ:ec * NQ + 512],
                                  in_=x_s[ec * 128:(ec + 1) * 128, 0:512])
            for ec in range(4):
                nc.sync.dma_start(out=wkt[:, ec * 512:ec * 512 + 128],
                                  in_=WkT[ec * 128:(ec + 1) * 128, 0:128])
            nc.sync.dma_start(out=ctx_sb[:, 0:NK], in_=ctx_s[0:128, :])
            nc.sync.dma_start(out=ctx_sb[:, NK:2 * NK], in_=ctx_s[128:256, :])
            for ec in range(4):
                nc.sync.dma_start(out=wkt[:, ec * 512 + 128:(ec + 1) * 512],
                                  in_=WkT[ec * 128:(ec + 1) * 128, 128:512])
            for ec in range(2):
                nc.sync.dma_start(out=x_sb[:, ec * NQ + 512:(ec + 1) * NQ],
                                  in_=x_s[ec * 128:(ec + 1) * 128, 512:NQ])
            for ec in range(4):
                nc.sync.dma_start(out=wot[:, ec * OC:(ec + 1) * OC],
                                  in_=WoT[ec * 128:(ec + 1) * 128, :])
            for ob in range(2):
                nc.sync.dma_start(out=bo_col[:, ob:ob + 1],
                                  in_=bo[ob * 128:(ob + 1) * 128])
            # gpsimd ring: ctx ec2/ec3, then wvt, then wqt rest
            nc.gpsimd.dma_start(out=ctx_sb[:, 2 * NK:3 * NK], in_=ctx_s[256:384, :])
            nc.gpsimd.dma_start(out=ctx_sb[:, 3 * NK:4 * NK], in_=ctx_s[384:512, :])
            for ec in range(4):
                nc.gpsimd.dma_start(out=wvt[:, ec * 512:(ec + 1) * 512],
                                    in_=WvT[ec * 128:(ec + 1) * 128, :])
            for ec in range(2):
                nc.gpsimd.dma_start(out=wqt[:, ec * 512 + 128:(ec + 1) * 512],
                                    in_=WqT[ec * 128:(ec + 1) * 128, 128:512])

            # persistent activations
            q_sb = qp.tile([128, 4 * NQ], F16, tag="q_sb")      # [hp, i]
            k_sb = qp.tile([128, 4 * NK], F16, tag="k_sb")      # [hp, j]
            vt_sb = qp.tile([128, JT * 520], F16, tag="vt_sb")  # [jt, h*65 + c]

            # ones columns of vt (col 64 of each 65-block)
            vt_4d = vt_sb.rearrange("p (j h c) -> p j h c", j=JT, h=H)
            ones_f32 = cp.tile([128, JT * H], F32, tag="ones_f32")
            nc.vector.memset(ones_f32, 1.0)
            nc.vector.tensor_copy(
                vt_4d[:, :, :, 64:65],
                ones_f32.rearrange("p (j h) -> p j h", j=JT).unsqueeze(-1))

            # ---- PE warm-up: dummy matmuls during the DMA wait ----
            # long enough to bridge into kproj0 so HAM stays at K=8/8
            for _ in range(10):
                wps = pp.tile([128, 512], F32, tag="pps")
                nc.tensor.matmul(wps, lhsT=warm_src[:, 0:128],
                                 rhs=warm_src, start=True, stop=True)

            # ---- projection emitters ----
            # rotate projection PSUM tiles through idle banks so the
            # single-buffer drain bubble never blocks the PE stream
            _rot = {"seq": []}

            def _ptile():
                if not _rot["seq"]:
                    _rot["seq"] = [(pp, "pps")]
                pool, tag = _rot["seq"].pop(0)
                t = pool.tile([128, 512], F32, tag=tag)
                return t

            def kproj_half(hp, ntile):
                pk = _ptile()
                for ec in range(4):
                    nc.tensor.matmul(
                        pk,
                        lhsT=wkt[:, ec * 512 + hp * 128: ec * 512 + (hp + 1) * 128],
                        rhs=ctx_sb[:, ec * NK + ntile * 512: ec * NK + (ntile + 1) * 512],
                        start=(ec == 0), stop=(ec == 3))
                nc.vector.tensor_copy(
                    k_sb[:, hp * NK + ntile * 512: hp * NK + (ntile + 1) * 512], pk)

            def vproj(jt):
                pv = _ptile()
                for ec in range(4):
                    nc.tensor.matmul(
                        pv,
                        lhsT=ctx_sb[:, ec * NK + jt * 128: ec * NK + (jt + 1) * 128],
                        rhs=wvt[:, ec * 512:(ec + 1) * 512],
                        start=(ec == 0), stop=(ec == 3))
                vt_t = vt_sb[:, jt * 520:(jt + 1) * 520].rearrange(
                    "p (h c) -> p h c", h=H)[:, :, 0:64]
                nc.vector.tensor_copy(vt_t, pv.rearrange("p (h c) -> p h c", c=64))

            def qproj(hp, it):
                pq = _ptile()
                for ec in range(2):
                    nc.tensor.matmul(
                        pq,
                        lhsT=wqt[:, ec * 512 + hp * 128: ec * 512 + (hp + 1) * 128],
                        rhs=x_sb[:, ec * NQ + it * 512: ec * NQ + (it + 1) * 512],
                        start=(ec == 0), stop=(ec == 1))
                nc.vector.tensor_copy(
                    q_sb[:, hp * NQ + it * 512: hp * NQ + (it + 1) * 512], pq)

            def sim_emit(hp, it, jt):
                slab = slabp.tile([128, 1024], F32, tag="slab")
                ks = slice(hp * NK + jt * 128, hp * NK + (jt + 1) * 128)
                qs = slice(hp * NQ + it * 512, hp * NQ + (it + 1) * 512)
                nc.tensor.matmul(
                    slab[:, 0:512], lhsT=k_sb[0:64, ks], rhs=q_sb[0:64, qs],
                    start=True, stop=True, tile_position=(0, 0))
                nc.tensor.matmul(
                    slab[:, 512:1024], lhsT=k_sb[64:128, ks], rhs=q_sb[64:128, qs],
                    start=True, stop=True, tile_position=(64, 0))
                return slab

            # ---- deferred-emission machinery ----
            # norm state per block b = it*4+hp
            st = {}

            def av_pair(b, jt):
                """AV accumulation for block b's exps at j-tile jt."""
                hp = b % 4
                if jt == 0:
                    a0 = avp.tile([128, 512], F32, tag="av0")
                    a1 = avp.tile([128, 512], F32, tag="av1")
                    st[b]["av"] = (a0, a1)
                av0, av1 = st[b]["av"]
                exps = st[b]["exps"][jt]
                nc.tensor.matmul(
                    av0[0:65, :],
                    lhsT=vt_sb[:, jt * 520 + (2 * hp) * 65: jt * 520 + (2 * hp) * 65 + 65],
                    rhs=exps[:, 0:512],
                    start=(jt == 0), stop=(jt == JT - 1))
                nc.tensor.matmul(
                    av1[0:65, :],
                    lhsT=vt_sb[:, jt * 520 + (2 * hp + 1) * 65: jt * 520 + (2 * hp + 1) * 65 + 65],
                    rhs=exps[:, 512:1024],
                    start=(jt == 0), stop=(jt == JT - 1))

            def norm_start(b):
                """After AV(b,7): drain AV banks, gather den rows (SBUF->SBUF)."""
                av0, av1 = st[b]["av"]
                avr = avrp.tile([65, 1024], F32, tag="avr")
                nc.vector.tensor_copy(avr[:, 0:512], av0[0:65, :])
                nc.vector.tensor_copy(avr[:, 512:1024], av1[0:65, :])
                stile = normp.tile([128, 8], F32, tag="stile")
                # b15's chain rides the scalar HWDGE ring so the tail chains
                # don't head-of-line block each other on the sync ring
                eng = nc.scalar if b >= 15 else nc.sync
                eng.dma_start(out=stile, in_=avr[64:65, :])
                st[b]["avr"] = avr
                st[b]["stile"] = stile

            def norm_recip(b):
                """reciprocal + scatter to DRAM + stride-0 broadcast reads."""
                it, hp = divmod(b, 4)
                stile_r = normp.tile([128, 8], F32, tag="stile_r")
                nc.vector.reciprocal(stile_r, st[b]["stile"])
                eng = nc.scalar if b >= 15 else nc.sync
                eng.dma_start(
                    out=sscratch2[it, hp].rearrange("(p f) -> p f", p=128),
                    in_=stile_r)
                sbc_a = normp.tile([64, 512], F32, tag="sbc_a")
                sbc_b = normp.tile([64, 512], F32, tag="sbc_b")
                eng.dma_start(
                    out=sbc_a,
                    in_=bass.AP(tensor=sscratch2, offset=b * 1024,
                                ap=[[0, 64], [1, 512]]))
                eng.dma_start(
                    out=sbc_b,
                    in_=bass.AP(tensor=sscratch2, offset=b * 1024 + 512,
                                ap=[[0, 64], [1, 512]]))
                st[b]["sbc"] = (sbc_a, sbc_b)

            def norm_mul(b):
                """normalized AV in f16."""
                avr = st[b]["avr"]
                sbc_a, sbc_b = st[b]["sbc"]
                avn = avnp.tile([128, 512], F16, tag="avn")
                nc.vector.tensor_mul(avn[0:64, :], avr[0:64, 0:512], sbc_a)
                nc.vector.tensor_mul(avn[64:128, :], avr[0:64, 512:1024], sbc_b)
                st[b]["avn"] = avn

            def oproj_mm(b):
                """ob0 (and for it3: ob1) matmul for cc=hp of block b's it."""
                it, hp = divmod(b, 4)
                if hp == 0:
                    y0_tile = yp.tile([128, 512], F32, tag="yps")
                    st[("y0", it)] = y0_tile
                    if it == 3:
                        y1_tile = pp.tile([128, 512], F32, tag="pps")
                        st[("y1", it)] = y1_tile
                nc.tensor.matmul(
                    st[("y0", it)],
                    lhsT=wot[:, hp * OC + 0 * 128: hp * OC + 1 * 128],
                    rhs=st[b]["avn"],
                    start=(hp == 0), stop=(hp == 3))
                if it == 3:
                    nc.tensor.matmul(
                        st[("y1", it)],
                        lhsT=wot[:, hp * OC + 1 * 128: hp * OC + 2 * 128],
                        rhs=st[b]["avn"],
                        start=(hp == 0), stop=(hp == 3))

            def oproj_fin(it):
                """drain ob0, run+drain ob1, store y."""
                yps = st[("y0", it)]
                ysb = yop.tile([128, 512], F32, tag="ysb")
                nc.vector.tensor_scalar_add(ysb, yps, bo_col[:, 0:1])
                seng = nc.scalar if it >= 2 else nc.sync
                seng.dma_start(
                    out=y[0:128, it * 512:(it + 1) * 512], in_=ysb)
                if it == 3:
                    yps1 = st[("y1", it)]
                else:
                    yps1 = yp.tile([128, 512], F32, tag="yps")
                    for cc in range(4):
                        nc.tensor.matmul(
                            yps1,
                            lhsT=wot[:, cc * OC + 1 * 128: cc * OC + 2 * 128],
                            rhs=st[it * 4 + cc]["avn"],
                            start=(cc == 0), stop=(cc == 3))
                ysb1 = yop.tile([128, 512], F32, tag="ysb")
                nc.vector.tensor_scalar_add(ysb1, yps1, bo_col[:, 1:2])
                seng.dma_start(
                    out=y[128:256, it * 512:(it + 1) * 512], in_=ysb1)

            # pending actions: (target_block, fn)
            pend = []

            def flush(b):
                keep = []
                for tb, fn in pend:
                    if tb <= b:
                        fn()
                    else:
                        keep.append((tb, fn))
                pend[:] = keep

            # per-block projection schedule: {block: {jt: [emitter, ...]}}
            # block 0 rotates through the idle av0/av1/pp banks; blocks 1-2
            # use pp+yp (yp is idle until block 4); later blocks only carry
            # one qproj via pp per block (ring reuse 8 steps apart = no
            # drain bubble on the PE stream).
            _rot["seq"] = [(avp, "av0"), (avp, "av1"), (pp, "pps")] * 3 + \
                          [(avp, "av0")] + \
                          [(pp, "pps"), (yp, "yps")] * 3 + [(pp, "pps")] * 40
            proj_sched = {
                0: {0: [lambda: kproj_half(1, 0)], 1: [lambda: kproj_half(1, 1)],
                    2: [lambda: qproj(1, 0)],
                    3: [lambda: vproj(1), lambda: vproj(2)],
                    4: [lambda: vproj(3), lambda: vproj(4)],
                    5: [lambda: vproj(5), lambda: vproj(6)],
                    6: [lambda: vproj(7)]},
                1: {2: [lambda: kproj_half(2, 0)], 3: [lambda: kproj_half(2, 1)],
                    5: [lambda: qproj(2, 0)]},
                2: {2: [lambda: kproj_half(3, 0)], 3: [lambda: kproj_half(3, 1)],
                    5: [lambda: qproj(3, 0)]},
            }
            # lazy qproj for block b+1 at step 4 of block b (b >= 3)
            for _b in range(3, 15):
                _it1, _hp1 = divmod(_b + 1, 4)
                proj_sched.setdefault(_b, {}).setdefault(4, []).append(
                    (lambda h, i: lambda: qproj(h, i))(_hp1, _it1))

            # ---- prologue projections (through the idle slab ring) ----
            # qproj first: its deps (x/wqt) are at the head of the load queue
            pqv = slabp.tile([128, 1024], F32, tag="slab")
            for ec in range(2):
                nc.tensor.matmul(
                    pqv[:, 0:512],
                    lhsT=wqt[:, ec * 512: ec * 512 + 128],
                    rhs=x_sb[:, ec * NQ: ec * NQ + 512],
                    start=(ec == 0), stop=(ec == 1))
            nc.vector.tensor_copy(q_sb[:, 0:512], pqv[:, 0:512])
            pk2 = slabp.tile([128, 1024], F32, tag="slab")
            for ntile in range(2):
                for ec in range(4):
                    nc.tensor.matmul(
                        pk2[:, ntile * 512:(ntile + 1) * 512],
                        lhsT=wkt[:, ec * 512: ec * 512 + 128],
                        rhs=ctx_sb[:, ec * NK + ntile * 512: ec * NK + (ntile + 1) * 512],
                        start=(ec == 0), stop=(ec == 3))
                nc.vector.tensor_copy(
                    k_sb[:, ntile * 512:(ntile + 1) * 512],
                    pk2[:, ntile * 512:(ntile + 1) * 512])
            vproj(0)

            # ---- main loop over blocks b = it*4 + hp ----
            # AVs for block b run during block b+1 (deferred), so projection
            # bubbles and AV-bank drains never stall the sim->exp stream.
            for b in range(16):
                it, hp = divmod(b, 4)
                st[b] = {"exps": []}
                flush(b)          # oproj MMs / oproj_fin due at this block

                slab = sim_emit(hp, it, 0)
                for jt in range(JT):
                    nslab = sim_emit(hp, it, jt + 1) if jt + 1 < JT else None
                    exps = ep.tile([128, 1024], F16, tag="exps")
                    nc.scalar.activation(exps, slab, EXP, bias=0.0, scale=SCALE)
                    st[b]["exps"].append(exps)
                    for fn in proj_sched.get(b, {}).get(jt, []):
                        fn()
                    if jt == 3:
                        # deferred DVE work, dependency-free by now
                        if b >= 2:
                            norm_recip(b - 2)
                        if b >= 3:
                            norm_mul(b - 3)
                    if b >= 1:
                        av_pair(b - 1, jt)
                    slab = nslab
                if b >= 1:
                    norm_start(b - 1)
                pend.append((b + 4, (lambda bb: lambda: oproj_mm(bb))(b)))
                if hp == 3:
                    pend.append((b + 5, (lambda ii: lambda: oproj_fin(ii))(it)))

            # block 15's own AVs: paced by the trailing exps on ACT
            for jt in range(JT):
                av_pair(15, jt)
            norm_start(15)

            # ---- tail: remaining recip/mul/oproj in dependency order ----
            # recip(15) before muls so the DVE in-order queue doesn't hold
            # the b15 chain behind b14's broadcast round-trip
            norm_recip(14)
            norm_recip(15)
            norm_mul(13)
            flush(16)            # fin(2), oproj_mm(12)
            norm_mul(14)
            flush(17)            # oproj_mm(13)
            norm_mul(15)
            flush(99)            # oproj_mm(14), oproj_mm(15), fin(3)

    _split_excess_waits(nc)
    return nc


_CACHED = None


def kernel(x, context, Wq, Wk, Wv, Wo, bo):
    global _CACHED
    if _CACHED is None:
        _CACHED = _build()
    nc = _CACHED

    x = np.asarray(x, dtype=np.float32)
    context = np.asarray(context, dtype=np.float32)
    xf = x.reshape(B, EQ, 64 * 64)
    cf = context.reshape(B, EK, 32 * 32)
    WqT = np.ascontiguousarray(np.asarray(Wq, np.float32).T.astype(np.float16))
    WkT = np.ascontiguousarray(np.asarray(Wk, np.float32).T.astype(np.float16))
    WvT = np.ascontiguousarray(np.asarray(Wv, np.float32).T.astype(np.float16))
    WoT = np.ascontiguousarray(np.asarray(Wo, np.float32).T.astype(np.float16))
    bo = np.ascontiguousarray(np.asarray(bo, np.float32))

    in_maps = []
    for core in range(8):
        b, half = core // 2, core % 2
        in_maps.append({
            "x_s": np.ascontiguousarray(xf[b, :, half * NQ:(half + 1) * NQ]).astype(np.float16),
            "ctx_s": np.ascontiguousarray(cf[b]).astype(np.float16),
            "WqT": WqT, "WkT": WkT, "WvT": WvT, "WoT": WoT, "bo": bo,
        })

    res = run_bass_kernel_spmd(nc, in_maps, list(range(8)))
    kernel.last_results = res

    out = np.empty((B, OC, 64 * 64), dtype=np.float32)
    for core in range(8):
        b, half = core // 2, core % 2
        out[b, :, half * NQ:(half + 1) * NQ] = res.results[core]["y"]
    return out.reshape(B, OC, 64, 64)
